# revision 37
# baseline (speedup 1.0000x reference)
"""Trainium2 Bass kernel for nn_ColorRenderer (SoftRas-style color renderer).

Algorithm (per pixel p, over faces f):
  score(f,p) = min(BIG*w0, BIG*w1, BIG*w2, -depth)   (affine banks in px,py)
  winner(p)  = argmax_f score;  valid(p) = maxscore > THRESH
  color(p)   = winner's affine color eval (host side), 0 if invalid.

v4: tile-culled rasterization, bank-major PSUM layout, K=9 matmuls.

The 256x256 screen is cut into 16x8-pixel tiles (one 128-partition chunk
each).  The host culls each tile's face list exactly; tiles with more
than 512 faces split into chunks.  Chunks are sorted by count and dealt
round-robin to the 8 cores (SPMD: identical instruction schedule, slot
face counts baked in, 32-face pad granularity).

Per slot (c faces): coef columns live bank-major [w0 c|w1 c|w2 c|nd c];
ceil(4c/512) matmuls with a single stationary [9,128] pixel operand
compute all banks into PSUM.  ACT drains [w1|nd] to SBUF, DVE does a
strided pair-min (min(w0,w1), min(w2,nd)), the final min, a max reduce,
and max_index.  The device returns per-slot (cmx, winner index); the
host evaluates the winner's color (pure per-pixel affine eval) and
scatters tiles into the frame.
"""

import numpy as np
import ml_dtypes

IMAGE_SIZE = 256
ORIG_SIZE = 512
DENOM_EPS = 1e-8

BIG = 1e14
THRESH = -5000.0
BAD = -3.0e30
NINIT = -3.0e38
NCORES = 8
TW, TH = 16, 8          # tile = 16x8 pixels = 128 partitions
CGRAN = 8               # face-count pad granularity
CMAX = 512              # max faces per slot (4c = 2048 psum cols = 4 banks)

bf16 = ml_dtypes.bfloat16

_PROGRAM_CACHE = {}


# ----------------------------------------------------------------------------
# Host-side math (projection, coefficients)
# ----------------------------------------------------------------------------

def _project_f32(vertices, K, R, t, dist_coeffs):
    """Faithful float32 replication of the reference projection."""
    f32 = np.float32
    EPS = f32(1e-9)
    v = np.einsum('bij,bvj->bvi', R.astype(f32), vertices.astype(f32)).astype(f32) + t.astype(f32)
    x, y, z = v[..., 0], v[..., 1], v[..., 2]
    x_ = (x / (z + EPS)).astype(f32)
    y_ = (y / (z + EPS)).astype(f32)
    r2 = (x_ * x_ + y_ * y_).astype(f32)
    d = dist_coeffs.astype(f32)
    k1 = d[:, 0:1]; k2 = d[:, 1:2]; p1 = d[:, 2:3]; p2 = d[:, 3:4]; k3 = d[:, 4:5]
    radial = (f32(1.0) + k1 * r2 + k2 * r2 ** 2 + k3 * r2 ** 3).astype(f32)
    x__ = (x_ * radial + f32(2.0) * p1 * x_ * y_ + p2 * (r2 + f32(2.0) * x_ * x_)).astype(f32)
    y__ = (y_ * radial + p1 * (r2 + f32(2.0) * y_ * y_) + f32(2.0) * p2 * x_ * y_).astype(f32)
    ones = np.ones_like(x__)
    uv = np.einsum('bij,bvj->bvi', K.astype(f32),
                   np.stack([x__, y__, ones], -1).astype(f32)).astype(f32)
    OS = f32(ORIG_SIZE)
    u = (f32(2.0) * (uv[..., 0] - OS / 2) / OS).astype(f32)
    vv = (f32(2.0) * ((OS - uv[..., 1]) - OS / 2) / OS).astype(f32)
    return np.stack([u, vv, z], -1).astype(f32)          # [B,V,3]


def _face_vertices_f32(verts, faces):
    f32 = np.float32
    IM = f32(IMAGE_SIZE)
    fv = verts[0][faces[0]]                               # [F,3,3]
    fv = fv * np.array([1.0, -1.0, 1.0], dtype=f32)
    fv = (fv * (IM / 2) + IM / 2).astype(f32)
    return fv


def _build_coeffs(fv):
    """Per-face f64 affine coefficients for w0,w1,w2,negdepth + ok mask."""
    f = fv.astype(np.float64)
    x0, y0, z0 = f[:, 0, 0], f[:, 0, 1], f[:, 0, 2]
    x1, y1, z1 = f[:, 1, 0], f[:, 1, 1], f[:, 1, 2]
    x2, y2, z2 = f[:, 2, 0], f[:, 2, 1], f[:, 2, 2]
    denom_f32 = ((fv[:, 1, 1] - fv[:, 2, 1]) * (fv[:, 0, 0] - fv[:, 2, 0])
                 + (fv[:, 2, 0] - fv[:, 1, 0]) * (fv[:, 0, 1] - fv[:, 2, 1])).astype(np.float32)
    ok = np.abs(denom_f32) > np.float32(DENOM_EPS)
    d = np.where(ok, (y1 - y2) * (x0 - x2) + (x2 - x1) * (y0 - y2), 1.0)
    a0 = (y1 - y2) / d; b0 = (x2 - x1) / d
    c0 = (-(y1 - y2) * x2 - (x2 - x1) * y2) / d
    a1 = (y2 - y0) / d; b1 = (x0 - x2) / d
    c1 = (-(y2 - y0) * x2 - (x0 - x2) * y2) / d
    a2 = (y0 - y1) / d; b2 = (x1 - x0) / d
    c2 = (-(y0 - y1) * x1 - (x1 - x0) * y1) / d
    and_ = -(a0 * z0 + a1 * z1 + a2 * z2)
    bnd = -(b0 * z0 + b1 * z1 + b2 * z2)
    cnd = -(c0 * z0 + c1 * z1 + c2 * z2)
    return dict(ok=ok, w0=(a0, b0, c0), w1=(a1, b1, c1), w2=(a2, b2, c2),
                nd=(and_, bnd, cnd))


def _split3_bf16(a):
    h = a.astype(bf16)
    r1 = a - h.astype(np.float64)
    m = r1.astype(bf16)
    l = (r1 - m.astype(np.float64)).astype(bf16)
    return h, m, l


def _bank_rows9(a, b, cc):
    """9 bf16 coefficient rows for one bank (c already recentered)."""
    ah, am, al = _split3_bf16(a)
    bh, bm, bl = _split3_bf16(b)
    ch, cm, cl = _split3_bf16(cc)
    return np.stack([ah, am, al, bh, bm, bl, ch, cm, cl], 0)   # [9, n] bf16


def _cull_tiles(fv, ok):
    """Exact-corner conservative cull: per 16x8 tile, faces overlapping it."""
    fxmin = fv[:, :, 0].min(1); fxmax = fv[:, :, 0].max(1)
    fymin = fv[:, :, 1].min(1); fymax = fv[:, :, 1].max(1)
    if not np.any(ok):
        return [], None
    xmin, xmax = fxmin[ok].min(), fxmax[ok].max()
    ymin, ymax = fymin[ok].min(), fymax[ok].max()
    c_lo = max(0, int(np.floor(xmin - 0.5)) - 1)
    c_hi = min(IMAGE_SIZE - 1, int(np.ceil(xmax - 0.5)) + 1)
    r_lo = max(0, int(np.floor(ymin - 0.5)) - 1)
    r_hi = min(IMAGE_SIZE - 1, int(np.ceil(ymax - 0.5)) + 1)
    if c_hi < c_lo or r_hi < r_lo:
        return [], None
    ntx = -(-(c_hi - c_lo + 1) // TW)
    nty = -(-(r_hi - r_lo + 1) // TH)
    f64 = fv.astype(np.float64)
    okidx = np.where(ok)[0]
    tiles = []
    for ty in range(nty):
        for tx in range(ntx):
            x0 = c_lo + tx * TW + 0.5; x1 = x0 + TW - 1
            y0 = r_lo + ty * TH + 0.5; y1 = y0 + TH - 1
            m = ((fxmax[okidx] >= x0) & (fxmin[okidx] <= x1)
                 & (fymax[okidx] >= y0) & (fymin[okidx] <= y1))
            idx = okidx[m]
            if len(idx) == 0:
                continue
            v = f64[idx]
            keep = np.ones(len(idx), bool)
            corners = np.array([[x0, y0], [x0, y1], [x1, y0], [x1, y1]])
            for e in range(3):
                a = v[:, e, :2]; b = v[:, (e + 1) % 3, :2]; c3 = v[:, (e + 2) % 3, :2]
                ex = b[:, 0] - a[:, 0]; ey = b[:, 1] - a[:, 1]
                win = ex * (c3[:, 1] - a[:, 1]) - ey * (c3[:, 0] - a[:, 0])
                wc = (ex[:, None] * (corners[None, :, 1] - a[:, None, 1])
                      - ey[:, None] * (corners[None, :, 0] - a[:, None, 0]))
                allout = np.all(wc * np.sign(win)[:, None] < -1e-9, axis=1)
                keep &= ~allout
            idx = idx[keep]
            if len(idx):
                tiles.append((ty, tx, idx))
    grid = dict(c_lo=c_lo, r_lo=r_lo, ntx=ntx, nty=nty)
    return tiles, grid


# ----------------------------------------------------------------------------
# Bass program
# ----------------------------------------------------------------------------

def _ffd_order(cpads):
    """First-fit-decreasing pack (sum c <= CMAX per bin) of desc-sorted
    cpads; returns the slot order with bins contiguous."""
    groups = []
    sums = []
    for s, c in enumerate(cpads):
        for gi, tot in enumerate(sums):
            if tot + c <= CMAX:
                groups[gi].append(s)
                sums[gi] += c
                break
        else:
            groups.append([s])
            sums.append(c)
    return [s for g in groups for s in g]


def _pack_groups(cpads):
    """Greedy scan of the (already FFD-ordered) cpads sequence into PSUM
    containers (sum c <= CMAX).  Shared by prepare() and _build_program()."""
    groups = []
    cur = []
    tot = 0
    for s, c in enumerate(cpads):
        if tot + c > CMAX:
            groups.append(cur)
            cur = []
            tot = 0
        cur.append(s)
        tot += c
    if cur:
        groups.append(cur)
    return groups


def _slot_order(cpads0):
    """Final slot order: a tiny seed container first (fast pipeline fill at
    cold PE p-state), a tiny exit container last (short serial tail), FFD
    bins in between with the fullest bins adjacent to seed/tail so the
    greedy re-scan cannot merge them."""
    n = len(cpads0)
    asc = sorted(range(n), key=lambda s: cpads0[s])
    if n < 6:
        return asc[::-1]
    seed = asc[:2]
    tail = asc[2:4]
    rest = [s for s in range(n) if s not in seed and s not in tail]
    rest.sort(key=lambda s: -cpads0[s])
    order1 = [rest[i] for i in _ffd_order([cpads0[s] for s in rest])]
    cp1 = [cpads0[o] for o in order1]
    bins = _pack_groups(cp1)
    bins.sort(key=lambda g: -sum(cp1[s] for s in g))
    # fullest bin right after seed and right before tail; the rest between
    mid = [order1[s] for g in bins[2:] for s in g] if len(bins) > 2 else []
    first = [order1[s] for s in bins[0]]
    last = [order1[s] for s in bins[1]] if len(bins) > 1 else []
    return seed + first + mid + last + tail


def _build_program(cpads):
    """cpads: tuple of per-slot padded face counts (multiples of CGRAN, <=CMAX)."""
    import concourse.bacc as bacc
    import concourse.tile as tile
    import concourse.bass as bass
    from concourse import mybir
    from contextlib import ExitStack

    S = len(cpads)
    groups = _pack_groups(cpads)
    gsum = [sum(cpads[s] for s in g) for g in groups]
    goff4 = np.concatenate([[0], np.cumsum([4 * t for t in gsum])]).astype(int)
    TOTC = int(goff4[-1])
    dt = mybir.dt
    op = mybir.AluOpType
    nc = bacc.Bacc("TRN2", target_bir_lowering=False, debug=False,
                   num_devices=NCORES)

    pixlhs = nc.dram_tensor("pixlhs", [9, 128], dt.bfloat16, kind="ExternalInput")
    # coefficient stream in 2 chunks so the first matmuls start early
    gsplit = max(1, len(groups) // 4)
    t0w = int(goff4[gsplit]); t1w = TOTC - t0w
    coefs0 = nc.dram_tensor("coefs0", [9, t0w], dt.bfloat16, kind="ExternalInput")
    coefs1 = nc.dram_tensor("coefs1", [9, max(t1w, 4)], dt.bfloat16, kind="ExternalInput")
    cmxout = nc.dram_tensor("cmxout", [128, S], dt.float32, kind="ExternalOutput")
    idxout = nc.dram_tensor("idxout", [128, S], dt.uint32, kind="ExternalOutput")

    with tile.TileContext(nc) as tc, ExitStack() as ctx:
        const = ctx.enter_context(tc.tile_pool(name="const", bufs=1))
        psum = ctx.enter_context(tc.tile_pool(name="psum", bufs=2, space="PSUM"))
        xyp = ctx.enter_context(tc.tile_pool(name="xyp", bufs=3))
        wnp = ctx.enter_context(tc.tile_pool(name="wnp", bufs=3))
        scp = ctx.enter_context(tc.tile_pool(name="scp", bufs=3))
        accp = ctx.enter_context(tc.tile_pool(name="accp", bufs=1))

        # spread input DMAs across engine queues so they issue in parallel
        pix_sb = const.tile([9, 128], dt.bfloat16)
        nc.scalar.dma_start(out=pix_sb[:], in_=pixlhs[:])
        coef_sb0 = const.tile([9, t0w], dt.bfloat16)
        nc.sync.dma_start(out=coef_sb0[:], in_=coefs0[:])
        coef_sb1 = const.tile([9, max(t1w, 4)], dt.bfloat16)
        nc.sync.dma_start(out=coef_sb1[:], in_=coefs1[:])

        cmx = accp.tile([128, S], dt.float32)
        idx8 = accp.tile([128, 8 * S], dt.uint32)
        idxc = accp.tile([128, S], dt.uint32)

        # warm the ACT table during the input DMAs so the first real drain
        # doesn't pay the ~1.3us ACT_TABLE_LOAD on the critical path
        warm = accp.tile([128, 8], dt.float32)
        nc.vector.memset(warm[:], 0.0)
        nc.scalar.copy(out=warm[:], in_=warm[:])

        def bAP(apv, dims, extra_off=0):
            return bass.AP(tensor=apv.tensor, offset=apv.offset + extra_off,
                           ap=[apv.ap[0]] + dims)

        for gi, g in enumerate(groups):
            T = gsum[gi]                     # total faces in this container
            if gi < gsplit:
                csb, coff = coef_sb0, int(goff4[gi])
            else:
                csb, coff = coef_sb1, int(goff4[gi] - t0w)
            P = psum.tile([128, 2048], dt.float32, tag="P")
            w = 4 * T
            for b in range(-(-w // 512)):
                lo = 512 * b
                hi = min(w, lo + 512)
                nc.tensor.matmul(
                    P[:, lo:hi], pix_sb[:, :], csb[:, coff + lo: coff + hi],
                    start=True, stop=True,
                )
            # ACT drains [W1|ND] to SBUF (TT cannot read 2 PSUM operands)
            p0 = P[:, 0:1]
            wn = wnp.tile([128, 1024], dt.float32, tag="wn")
            w0_ = wn[:, 0:1]
            nc.scalar.copy(
                out=bAP(w0_, [[T, 2], [1, T]]),
                in_=bAP(p0, [[2 * T, 2], [1, T]], extra_off=T))
            # L1: xy[128, 2, T] = min([W0|W2] (PSUM), [W1|ND] (SBUF))
            xy = xyp.tile([128, 1024], dt.float32, tag="xy")
            x0 = xy[:, 0:1]
            nc.vector.tensor_tensor(
                out=bAP(x0, [[T, 2], [1, T]]),
                in0=bAP(p0, [[2 * T, 2], [1, T]]),
                in1=bAP(w0_, [[T, 2], [1, T]]),
                op=op.min)
            # L2: score = min(m01, m2d)   (slot-contiguous sections)
            score = scp.tile([128, CMAX], dt.float32, tag="score")
            nc.vector.tensor_tensor(out=score[:, 0:T], in0=xy[:, 0:T],
                                    in1=xy[:, T:2 * T], op=op.min)
            # per-slot reduce (batched over runs of equal-c slots) + argmax
            off = 0
            j = 0
            while j < len(g):
                s0 = g[j]
                c = cpads[s0]
                q = 1
                while (j + q < len(g) and cpads[g[j + q]] == c
                       and g[j + q] == s0 + q):
                    q += 1
                cm0 = cmx[:, s0:s0 + 1]
                if q == 1:
                    nc.vector.tensor_reduce(out=cm0, in_=score[:, off:off + c],
                                            axis=mybir.AxisListType.X, op=op.max)
                else:
                    sc0 = score[:, off:off + 1]
                    nc.vector.tensor_reduce(
                        out=bAP(cm0, [[1, q]]),
                        in_=bAP(sc0, [[c, q], [1, c]]),
                        axis=mybir.AxisListType.X, op=op.max)
                for i in range(q):
                    s = g[j + i]
                    cm = cmx[:, s:s + 1]
                    cm8 = bass.AP(tensor=cm.tensor, offset=cm.offset,
                                  ap=[cm.ap[0], [0, 8]])
                    nc.vector.max_index(out=idx8[:, 8 * s:8 * s + 8],
                                        in_max=cm8,
                                        in_values=score[:, off:off + c])
                    off += c
                j += q

        nc.scalar.dma_start(out=cmxout[:], in_=cmx[:])
        i0 = idx8[:, 0:1]
        nc.vector.tensor_copy(out=idxc[:], in_=bAP(i0, [[8, S]]))
        nc.sync.dma_start(out=idxout[:], in_=idxc[:])

    nc.compile()
    return nc


def _get_program(cpads):
    key = tuple(cpads)
    if key not in _PROGRAM_CACHE:
        _PROGRAM_CACHE[key] = _build_program(key)
    return _PROGRAM_CACHE[key]


# ----------------------------------------------------------------------------
# Host orchestration
# ----------------------------------------------------------------------------

def prepare(vertices, faces, textures, K, R, t, dist_coeffs):
    """All host-side prep.  Returns (cpads, in_maps, scatter)."""
    verts = _project_f32(np.asarray(vertices), np.asarray(K), np.asarray(R),
                         np.asarray(t), np.asarray(dist_coeffs))
    fv = _face_vertices_f32(verts, np.asarray(faces))
    co = _build_coeffs(fv)
    tiles, grid = _cull_tiles(fv, co['ok'])
    if not tiles:
        return None
    tex = np.asarray(textures)[0].astype(np.float64)      # [F,3,C]

    # color affine coefficients per face (global coords)  [F, 9] f64
    F = fv.shape[0]
    colABC = np.zeros((F, 9), dtype=np.float64)
    for ch in range(3):
        t0, t1, t2 = tex[:, 0, ch], tex[:, 1, ch], tex[:, 2, ch]
        colABC[:, 3 * ch + 0] = co['w0'][0] * t0 + co['w1'][0] * t1 + co['w2'][0] * t2
        colABC[:, 3 * ch + 1] = co['w0'][1] * t0 + co['w1'][1] * t1 + co['w2'][1] * t2
        colABC[:, 3 * ch + 2] = co['w0'][2] * t0 + co['w1'][2] * t1 + co['w2'][2] * t2

    # split big tiles into chunks <= CMAX, keeping a tile id for host merge
    chunks = []           # (tile_id, ty, tx, fidx)
    for tid, (ty, tx, fidx) in enumerate(tiles):
        n = len(fidx)
        nch = -(-n // CMAX)
        per = -(-n // nch)
        for j in range(0, n, per):
            chunks.append((tid, ty, tx, fidx[j:j + per]))

    # sort chunks by count desc, deal round-robin; slot pad = octet max
    chunks.sort(key=lambda ch: -len(ch[3]))
    nchunk = len(chunks)
    S = -(-nchunk // NCORES)
    cpads0 = []
    for s in range(S):
        grp = chunks[8 * s: 8 * s + 8]
        cmax = max(len(ch[3]) for ch in grp)
        cpads0.append(max(CGRAN, -(-cmax // CGRAN) * CGRAN))
    # reorder slots so PSUM containers are contiguous slot ranges
    order = _slot_order(cpads0)
    cpads = [cpads0[o] for o in order]
    groups = _pack_groups(cpads)
    gsum = [sum(cpads[s] for s in g) for g in groups]
    goff4 = np.concatenate([[0], np.cumsum([4 * t for t in gsum])]).astype(int)
    TOTC = int(goff4[-1])
    gsplit = max(1, len(groups) // 4)
    t0w = int(goff4[gsplit])
    # per-slot (group, bank-offset-base, within-group offset)
    slot_place = {}
    for gi, gr in enumerate(groups):
        off = 0
        for s in gr:
            slot_place[s] = (gi, int(goff4[gi]), off, gsum[gi])
            off += cpads[s]

    pp = np.arange(128)
    pxl = (pp % TW) - (TW / 2 - 0.5)          # -7.5 .. 7.5
    pyl = (pp // TW) - (TH / 2 - 0.5)         # -3.5 .. 3.5

    # stationary matmul operand [9,128]: rows [px*3, py*3, 1*3]
    pixlhs = np.stack([pxl, pxl, pxl, pyl, pyl, pyl,
                       np.ones(128), np.ones(128), np.ones(128)]).astype(bf16)
    assert np.all(pixlhs[0].astype(np.float64) == pxl)
    assert np.all(pixlhs[3].astype(np.float64) == pyl)

    c_lo, r_lo = grid['c_lo'], grid['r_lo']
    banks = ['w0', 'w1', 'w2', 'nd']

    in_maps = []
    rows_of = np.zeros((NCORES, S, 128), dtype=np.int32)
    cols_of = np.zeros((NCORES, S, 128), dtype=np.int32)
    real_of = np.zeros((NCORES, S, 128), dtype=bool)
    tile_of = np.full((NCORES, S), -1, dtype=np.int32)
    faces_of = [[None] * S for _ in range(NCORES)]
    for k in range(NCORES):
        coefs = np.zeros((9, TOTC), dtype=bf16)
        for s in range(S):
            c = cpads[s]
            gi, gbase, goff, gtot = slot_place[s]
            ci = 8 * order[s] + k
            if ci < nchunk:
                tid, ty, tx, fidx = chunks[ci]
                n = len(fidx)
                sx = c_lo + tx * TW + TW / 2.0
                sy = r_lo + ty * TH + TH / 2.0
                gx = c_lo + tx * TW + (pp % TW)
                gy = r_lo + ty * TH + (pp // TW)
                real = (gx <= IMAGE_SIZE - 1) & (gy <= IMAGE_SIZE - 1)
                rows_of[k, s] = np.minimum(gy, IMAGE_SIZE - 1)
                cols_of[k, s] = np.minimum(gx, IMAGE_SIZE - 1)
                real_of[k, s] = real
                tile_of[k, s] = tid
                faces_of[k][s] = fidx
            else:
                n = 0
            # bank-major coefficient columns [w0 c | w1 c | w2 c | nd c]
            for g in range(4):
                if g < 3:
                    a, b, cc = (v.copy() for v in co[banks[g]])
                    a = a * BIG; b = b * BIG; cc = cc * BIG
                else:
                    a, b, cc = (v.copy() for v in co['nd'])
                if n:
                    av = a[fidx]; bv = b[fidx]
                    cv = cc[fidx] + av * sx + bv * sy
                    av = np.concatenate([av, np.zeros(c - n)])
                    bv = np.concatenate([bv, np.zeros(c - n)])
                    cv = np.concatenate([cv, np.full(c - n, BAD if g == 3 else 0.0)])
                else:
                    av = np.zeros(c); bv = np.zeros(c)
                    cv = np.full(c, BAD if g == 3 else 0.0)
                lo = gbase + g * gtot + goff
                coefs[:, lo: lo + c] = _bank_rows9(av, bv, cv)
        im = dict(pixlhs=pixlhs, coefs0=coefs[:, :t0w],
                  coefs1=coefs[:, t0w:] if t0w < TOTC else np.zeros((9, 4), dtype=bf16))
        if im["coefs1"].shape[1] < 4:
            pad = np.zeros((9, 4), dtype=bf16)
            pad[:, :im["coefs1"].shape[1]] = im["coefs1"]
            im["coefs1"] = pad
        in_maps.append(im)

    scatter = dict(rows_of=rows_of, cols_of=cols_of, real_of=real_of,
                   tile_of=tile_of, faces_of=faces_of, colABC=colABC,
                   S=S, nchunk=nchunk)
    return cpads, in_maps, scatter


def assemble(results, scatter):
    out = np.zeros((1, 3, IMAGE_SIZE, IMAGE_SIZE), dtype=np.float32)
    S = scatter['S']
    tile_of = scatter['tile_of']
    colABC = scatter['colABC']
    bestcmx = np.full((IMAGE_SIZE, IMAGE_SIZE), -np.inf, dtype=np.float32)
    for k in range(NCORES):
        cmx = results[k]['cmxout']                        # [128, S]
        idx = results[k]['idxout']                        # [128, S]
        for s in range(S):
            if tile_of[k, s] < 0:
                continue
            fidx = scatter['faces_of'][k][s]
            n = len(fidx)
            valid = (cmx[:, s] > THRESH) & scatter['real_of'][k, s]
            if not np.any(valid):
                continue
            wi = np.minimum(idx[valid, s].astype(np.int64), n - 1)
            fglob = fidx[wi]
            rr = scatter['rows_of'][k, s][valid]
            cc = scatter['cols_of'][k, s][valid]
            cm = cmx[valid, s]
            upd = cm > bestcmx[rr, cc]
            if not np.any(upd):
                continue
            rr = rr[upd]; cc = cc[upd]
            bestcmx[rr, cc] = cm[upd]
            A = colABC[fglob[upd]]                        # [m, 9]
            px = cc + 0.5
            py = rr + 0.5
            for ch in range(3):
                out[0, ch, rr, cc] = (A[:, 3 * ch] * px + A[:, 3 * ch + 1] * py
                                      + A[:, 3 * ch + 2]).astype(np.float32)
    return out


def kernel(**inputs):
    from concourse.bass_utils import run_bass_kernel_spmd

    prep = prepare(**inputs)
    if prep is None:
        return np.zeros((1, 3, IMAGE_SIZE, IMAGE_SIZE), dtype=np.float32)
    cpads, in_maps, scatter = prep
    nc = _get_program(cpads)
    res = run_bass_kernel_spmd(nc, in_maps, core_ids=list(range(NCORES)))
    return assemble(res.results, scatter)


if __name__ == "__main__":
    pass


# revision 38
# speedup vs baseline: 1.0054x; 1.0054x over previous
"""Trainium2 Bass kernel for nn_ColorRenderer (SoftRas-style color renderer).

Algorithm (per pixel p, over faces f):
  score(f,p) = min(BIG*w0, BIG*w1, BIG*w2, -depth)   (affine banks in px,py)
  winner(p)  = argmax_f score;  valid(p) = maxscore > THRESH
  color(p)   = winner's affine color eval (host side), 0 if invalid.

v4: tile-culled rasterization, bank-major PSUM layout, K=9 matmuls.

The 256x256 screen is cut into 16x8-pixel tiles (one 128-partition chunk
each).  The host culls each tile's face list exactly; tiles with more
than 512 faces split into chunks.  Chunks are sorted by count and dealt
round-robin to the 8 cores (SPMD: identical instruction schedule, slot
face counts baked in, 32-face pad granularity).

Per slot (c faces): coef columns live bank-major [w0 c|w1 c|w2 c|nd c];
ceil(4c/512) matmuls with a single stationary [9,128] pixel operand
compute all banks into PSUM.  ACT drains [w1|nd] to SBUF, DVE does a
strided pair-min (min(w0,w1), min(w2,nd)), the final min, a max reduce,
and max_index.  The device returns per-slot (cmx, winner index); the
host evaluates the winner's color (pure per-pixel affine eval) and
scatters tiles into the frame.
"""

import numpy as np
import ml_dtypes

IMAGE_SIZE = 256
ORIG_SIZE = 512
DENOM_EPS = 1e-8

BIG = 1e14
THRESH = -5000.0
BAD = -3.0e30
NINIT = -3.0e38
NCORES = 8
TW, TH = 16, 8          # tile = 16x8 pixels = 128 partitions
CGRAN = 8               # face-count pad granularity
CMAX = 512              # max faces per slot (4c = 2048 psum cols = 4 banks)

bf16 = ml_dtypes.bfloat16

_PROGRAM_CACHE = {}


# ----------------------------------------------------------------------------
# Host-side math (projection, coefficients)
# ----------------------------------------------------------------------------

def _project_f32(vertices, K, R, t, dist_coeffs):
    """Faithful float32 replication of the reference projection."""
    f32 = np.float32
    EPS = f32(1e-9)
    v = np.einsum('bij,bvj->bvi', R.astype(f32), vertices.astype(f32)).astype(f32) + t.astype(f32)
    x, y, z = v[..., 0], v[..., 1], v[..., 2]
    x_ = (x / (z + EPS)).astype(f32)
    y_ = (y / (z + EPS)).astype(f32)
    r2 = (x_ * x_ + y_ * y_).astype(f32)
    d = dist_coeffs.astype(f32)
    k1 = d[:, 0:1]; k2 = d[:, 1:2]; p1 = d[:, 2:3]; p2 = d[:, 3:4]; k3 = d[:, 4:5]
    radial = (f32(1.0) + k1 * r2 + k2 * r2 ** 2 + k3 * r2 ** 3).astype(f32)
    x__ = (x_ * radial + f32(2.0) * p1 * x_ * y_ + p2 * (r2 + f32(2.0) * x_ * x_)).astype(f32)
    y__ = (y_ * radial + p1 * (r2 + f32(2.0) * y_ * y_) + f32(2.0) * p2 * x_ * y_).astype(f32)
    ones = np.ones_like(x__)
    uv = np.einsum('bij,bvj->bvi', K.astype(f32),
                   np.stack([x__, y__, ones], -1).astype(f32)).astype(f32)
    OS = f32(ORIG_SIZE)
    u = (f32(2.0) * (uv[..., 0] - OS / 2) / OS).astype(f32)
    vv = (f32(2.0) * ((OS - uv[..., 1]) - OS / 2) / OS).astype(f32)
    return np.stack([u, vv, z], -1).astype(f32)          # [B,V,3]


def _face_vertices_f32(verts, faces):
    f32 = np.float32
    IM = f32(IMAGE_SIZE)
    fv = verts[0][faces[0]]                               # [F,3,3]
    fv = fv * np.array([1.0, -1.0, 1.0], dtype=f32)
    fv = (fv * (IM / 2) + IM / 2).astype(f32)
    return fv


def _build_coeffs(fv):
    """Per-face f64 affine coefficients for w0,w1,w2,negdepth + ok mask."""
    f = fv.astype(np.float64)
    x0, y0, z0 = f[:, 0, 0], f[:, 0, 1], f[:, 0, 2]
    x1, y1, z1 = f[:, 1, 0], f[:, 1, 1], f[:, 1, 2]
    x2, y2, z2 = f[:, 2, 0], f[:, 2, 1], f[:, 2, 2]
    denom_f32 = ((fv[:, 1, 1] - fv[:, 2, 1]) * (fv[:, 0, 0] - fv[:, 2, 0])
                 + (fv[:, 2, 0] - fv[:, 1, 0]) * (fv[:, 0, 1] - fv[:, 2, 1])).astype(np.float32)
    ok = np.abs(denom_f32) > np.float32(DENOM_EPS)
    d = np.where(ok, (y1 - y2) * (x0 - x2) + (x2 - x1) * (y0 - y2), 1.0)
    a0 = (y1 - y2) / d; b0 = (x2 - x1) / d
    c0 = (-(y1 - y2) * x2 - (x2 - x1) * y2) / d
    a1 = (y2 - y0) / d; b1 = (x0 - x2) / d
    c1 = (-(y2 - y0) * x2 - (x0 - x2) * y2) / d
    a2 = (y0 - y1) / d; b2 = (x1 - x0) / d
    c2 = (-(y0 - y1) * x1 - (x1 - x0) * y1) / d
    and_ = -(a0 * z0 + a1 * z1 + a2 * z2)
    bnd = -(b0 * z0 + b1 * z1 + b2 * z2)
    cnd = -(c0 * z0 + c1 * z1 + c2 * z2)
    return dict(ok=ok, w0=(a0, b0, c0), w1=(a1, b1, c1), w2=(a2, b2, c2),
                nd=(and_, bnd, cnd))


def _split3_bf16(a):
    h = a.astype(bf16)
    r1 = a - h.astype(np.float64)
    m = r1.astype(bf16)
    l = (r1 - m.astype(np.float64)).astype(bf16)
    return h, m, l


def _bank_rows9(a, b, cc):
    """9 bf16 coefficient rows for one bank (c already recentered)."""
    ah, am, al = _split3_bf16(a)
    bh, bm, bl = _split3_bf16(b)
    ch, cm, cl = _split3_bf16(cc)
    return np.stack([ah, am, al, bh, bm, bl, ch, cm, cl], 0)   # [9, n] bf16


def _cull_tiles(fv, ok):
    """Exact-corner conservative cull: per 16x8 tile, faces overlapping it."""
    fxmin = fv[:, :, 0].min(1); fxmax = fv[:, :, 0].max(1)
    fymin = fv[:, :, 1].min(1); fymax = fv[:, :, 1].max(1)
    if not np.any(ok):
        return [], None
    xmin, xmax = fxmin[ok].min(), fxmax[ok].max()
    ymin, ymax = fymin[ok].min(), fymax[ok].max()
    c_lo = max(0, int(np.floor(xmin - 0.5)) - 1)
    c_hi = min(IMAGE_SIZE - 1, int(np.ceil(xmax - 0.5)) + 1)
    r_lo = max(0, int(np.floor(ymin - 0.5)) - 1)
    r_hi = min(IMAGE_SIZE - 1, int(np.ceil(ymax - 0.5)) + 1)
    if c_hi < c_lo or r_hi < r_lo:
        return [], None
    ntx = -(-(c_hi - c_lo + 1) // TW)
    nty = -(-(r_hi - r_lo + 1) // TH)
    f64 = fv.astype(np.float64)
    okidx = np.where(ok)[0]
    tiles = []
    for ty in range(nty):
        for tx in range(ntx):
            x0 = c_lo + tx * TW + 0.5; x1 = x0 + TW - 1
            y0 = r_lo + ty * TH + 0.5; y1 = y0 + TH - 1
            m = ((fxmax[okidx] >= x0) & (fxmin[okidx] <= x1)
                 & (fymax[okidx] >= y0) & (fymin[okidx] <= y1))
            idx = okidx[m]
            if len(idx) == 0:
                continue
            v = f64[idx]
            keep = np.ones(len(idx), bool)
            corners = np.array([[x0, y0], [x0, y1], [x1, y0], [x1, y1]])
            for e in range(3):
                a = v[:, e, :2]; b = v[:, (e + 1) % 3, :2]; c3 = v[:, (e + 2) % 3, :2]
                ex = b[:, 0] - a[:, 0]; ey = b[:, 1] - a[:, 1]
                win = ex * (c3[:, 1] - a[:, 1]) - ey * (c3[:, 0] - a[:, 0])
                wc = (ex[:, None] * (corners[None, :, 1] - a[:, None, 1])
                      - ey[:, None] * (corners[None, :, 0] - a[:, None, 0]))
                allout = np.all(wc * np.sign(win)[:, None] < -1e-9, axis=1)
                keep &= ~allout
            idx = idx[keep]
            if len(idx):
                tiles.append((ty, tx, idx))
    grid = dict(c_lo=c_lo, r_lo=r_lo, ntx=ntx, nty=nty)
    return tiles, grid


# ----------------------------------------------------------------------------
# Bass program
# ----------------------------------------------------------------------------

def _ffd_order(cpads):
    """First-fit-decreasing pack (sum c <= CMAX per bin) of desc-sorted
    cpads; returns the slot order with bins contiguous."""
    groups = []
    sums = []
    for s, c in enumerate(cpads):
        for gi, tot in enumerate(sums):
            if tot + c <= CMAX:
                groups[gi].append(s)
                sums[gi] += c
                break
        else:
            groups.append([s])
            sums.append(c)
    return [s for g in groups for s in g]


def _pack_groups(cpads):
    """Greedy scan of the (already FFD-ordered) cpads sequence into PSUM
    containers (sum c <= CMAX).  Shared by prepare() and _build_program()."""
    groups = []
    cur = []
    tot = 0
    for s, c in enumerate(cpads):
        if tot + c > CMAX:
            groups.append(cur)
            cur = []
            tot = 0
        cur.append(s)
        tot += c
    if cur:
        groups.append(cur)
    return groups


def _slot_order(cpads0):
    """Final slot order: a tiny seed container first (fast pipeline fill at
    cold PE p-state), a tiny exit container last (short serial tail), FFD
    bins in between with the fullest bins adjacent to seed/tail so the
    greedy re-scan cannot merge them."""
    n = len(cpads0)
    asc = sorted(range(n), key=lambda s: cpads0[s])
    if n < 6:
        return asc[::-1]
    seed = asc[:2]
    tail = asc[2:4]
    rest = [s for s in range(n) if s not in seed and s not in tail]
    rest.sort(key=lambda s: -cpads0[s])
    order1 = [rest[i] for i in _ffd_order([cpads0[s] for s in rest])]
    cp1 = [cpads0[o] for o in order1]
    bins = _pack_groups(cp1)
    bins.sort(key=lambda g: -sum(cp1[s] for s in g))
    # fullest bin right after seed and right before tail; the rest between
    mid = [order1[s] for g in bins[2:] for s in g] if len(bins) > 2 else []
    first = [order1[s] for s in bins[0]]
    last = [order1[s] for s in bins[1]] if len(bins) > 1 else []
    return seed + first + mid + last + tail


def _build_program(cpads):
    """cpads: tuple of per-slot padded face counts (multiples of CGRAN, <=CMAX)."""
    import concourse.bacc as bacc
    import concourse.tile as tile
    import concourse.bass as bass
    from concourse import mybir
    from contextlib import ExitStack

    S = len(cpads)
    groups = _pack_groups(cpads)
    gsum = [sum(cpads[s] for s in g) for g in groups]
    goff4 = np.concatenate([[0], np.cumsum([4 * t for t in gsum])]).astype(int)
    TOTC = int(goff4[-1])
    dt = mybir.dt
    op = mybir.AluOpType
    nc = bacc.Bacc("TRN2", target_bir_lowering=False, debug=False,
                   num_devices=NCORES)

    pixlhs = nc.dram_tensor("pixlhs", [9, 128], dt.bfloat16, kind="ExternalInput")
    # coefficient stream in 2 chunks so the first matmuls start early
    gsplit = max(1, len(groups) // 4)
    t0w = int(goff4[gsplit]); t1w = TOTC - t0w
    coefs0 = nc.dram_tensor("coefs0", [9, t0w], dt.bfloat16, kind="ExternalInput")
    coefs1 = nc.dram_tensor("coefs1", [9, max(t1w, 4)], dt.bfloat16, kind="ExternalInput")
    cmxout = nc.dram_tensor("cmxout", [128, S], dt.float32, kind="ExternalOutput")
    idxout = nc.dram_tensor("idxout", [128, S], dt.uint32, kind="ExternalOutput")

    with tile.TileContext(nc) as tc, ExitStack() as ctx:
        const = ctx.enter_context(tc.tile_pool(name="const", bufs=1))
        psum = ctx.enter_context(tc.tile_pool(name="psum", bufs=2, space="PSUM"))
        xyp = ctx.enter_context(tc.tile_pool(name="xyp", bufs=3))
        wnp = ctx.enter_context(tc.tile_pool(name="wnp", bufs=3))
        scp = ctx.enter_context(tc.tile_pool(name="scp", bufs=3))
        accp = ctx.enter_context(tc.tile_pool(name="accp", bufs=1))

        # spread input DMAs across engine queues so they issue in parallel
        pix_sb = const.tile([9, 128], dt.bfloat16)
        nc.scalar.dma_start(out=pix_sb[:], in_=pixlhs[:])
        coef_sb0 = const.tile([9, t0w], dt.bfloat16)
        nc.sync.dma_start(out=coef_sb0[:], in_=coefs0[:])
        coef_sb1 = const.tile([9, max(t1w, 4)], dt.bfloat16)
        nc.sync.dma_start(out=coef_sb1[:], in_=coefs1[:])

        cmx = accp.tile([128, S], dt.float32)
        idx8 = accp.tile([128, 8 * S], dt.uint32)
        idxc = accp.tile([128, S], dt.uint32)

        # warm the ACT table during the input DMAs so the first real drain
        # doesn't pay the ~1.3us ACT_TABLE_LOAD on the critical path
        warm = accp.tile([128, 8], dt.float32)
        nc.vector.memset(warm[:], 0.0)
        nc.scalar.copy(out=warm[:], in_=warm[:])
        # keep PE continuously busy while the coef DMAs are in flight so its
        # p-state ramps (0.65 -> 1.2 -> 2.4GHz needs ~3us of busy); the dummy
        # output is never read
        wmm = const.tile([9, 512], dt.bfloat16)
        nc.gpsimd.memset(wmm[:], 0.0)
        Pw = psum.tile([128, 2048], dt.float32, tag="P")
        for b in range(4):
            nc.tensor.matmul(Pw[:, 512 * b:512 * (b + 1)], wmm[:, 0:128],
                             wmm[:, 0:512], start=True, stop=True)

        def bAP(apv, dims, extra_off=0):
            return bass.AP(tensor=apv.tensor, offset=apv.offset + extra_off,
                           ap=[apv.ap[0]] + dims)

        for gi, g in enumerate(groups):
            T = gsum[gi]                     # total faces in this container
            if gi < gsplit:
                csb, coff = coef_sb0, int(goff4[gi])
            else:
                csb, coff = coef_sb1, int(goff4[gi] - t0w)
            P = psum.tile([128, 2048], dt.float32, tag="P")
            w = 4 * T
            for b in range(-(-w // 512)):
                lo = 512 * b
                hi = min(w, lo + 512)
                nc.tensor.matmul(
                    P[:, lo:hi], pix_sb[:, :], csb[:, coff + lo: coff + hi],
                    start=True, stop=True,
                )
            # ACT drains [W1|ND] to SBUF (TT cannot read 2 PSUM operands)
            p0 = P[:, 0:1]
            wn = wnp.tile([128, 1024], dt.float32, tag="wn")
            w0_ = wn[:, 0:1]
            nc.scalar.copy(
                out=bAP(w0_, [[T, 2], [1, T]]),
                in_=bAP(p0, [[2 * T, 2], [1, T]], extra_off=T))
            # L1: xy[128, 2, T] = min([W0|W2] (PSUM), [W1|ND] (SBUF))
            xy = xyp.tile([128, 1024], dt.float32, tag="xy")
            x0 = xy[:, 0:1]
            nc.vector.tensor_tensor(
                out=bAP(x0, [[T, 2], [1, T]]),
                in0=bAP(p0, [[2 * T, 2], [1, T]]),
                in1=bAP(w0_, [[T, 2], [1, T]]),
                op=op.min)
            # L2: score = min(m01, m2d)   (slot-contiguous sections)
            score = scp.tile([128, CMAX], dt.float32, tag="score")
            nc.vector.tensor_tensor(out=score[:, 0:T], in0=xy[:, 0:T],
                                    in1=xy[:, T:2 * T], op=op.min)
            # per-slot reduce (batched over runs of equal-c slots) + argmax
            off = 0
            j = 0
            while j < len(g):
                s0 = g[j]
                c = cpads[s0]
                q = 1
                while (j + q < len(g) and cpads[g[j + q]] == c
                       and g[j + q] == s0 + q):
                    q += 1
                cm0 = cmx[:, s0:s0 + 1]
                if q == 1:
                    nc.vector.tensor_reduce(out=cm0, in_=score[:, off:off + c],
                                            axis=mybir.AxisListType.X, op=op.max)
                else:
                    sc0 = score[:, off:off + 1]
                    nc.vector.tensor_reduce(
                        out=bAP(cm0, [[1, q]]),
                        in_=bAP(sc0, [[c, q], [1, c]]),
                        axis=mybir.AxisListType.X, op=op.max)
                for i in range(q):
                    s = g[j + i]
                    cm = cmx[:, s:s + 1]
                    cm8 = bass.AP(tensor=cm.tensor, offset=cm.offset,
                                  ap=[cm.ap[0], [0, 8]])
                    nc.vector.max_index(out=idx8[:, 8 * s:8 * s + 8],
                                        in_max=cm8,
                                        in_values=score[:, off:off + c])
                    off += c
                j += q

        nc.scalar.dma_start(out=cmxout[:], in_=cmx[:])
        i0 = idx8[:, 0:1]
        nc.vector.tensor_copy(out=idxc[:], in_=bAP(i0, [[8, S]]))
        nc.sync.dma_start(out=idxout[:], in_=idxc[:])

    nc.compile()
    return nc


def _get_program(cpads):
    key = tuple(cpads)
    if key not in _PROGRAM_CACHE:
        _PROGRAM_CACHE[key] = _build_program(key)
    return _PROGRAM_CACHE[key]


# ----------------------------------------------------------------------------
# Host orchestration
# ----------------------------------------------------------------------------

def prepare(vertices, faces, textures, K, R, t, dist_coeffs):
    """All host-side prep.  Returns (cpads, in_maps, scatter)."""
    verts = _project_f32(np.asarray(vertices), np.asarray(K), np.asarray(R),
                         np.asarray(t), np.asarray(dist_coeffs))
    fv = _face_vertices_f32(verts, np.asarray(faces))
    co = _build_coeffs(fv)
    tiles, grid = _cull_tiles(fv, co['ok'])
    if not tiles:
        return None
    tex = np.asarray(textures)[0].astype(np.float64)      # [F,3,C]

    # color affine coefficients per face (global coords)  [F, 9] f64
    F = fv.shape[0]
    colABC = np.zeros((F, 9), dtype=np.float64)
    for ch in range(3):
        t0, t1, t2 = tex[:, 0, ch], tex[:, 1, ch], tex[:, 2, ch]
        colABC[:, 3 * ch + 0] = co['w0'][0] * t0 + co['w1'][0] * t1 + co['w2'][0] * t2
        colABC[:, 3 * ch + 1] = co['w0'][1] * t0 + co['w1'][1] * t1 + co['w2'][1] * t2
        colABC[:, 3 * ch + 2] = co['w0'][2] * t0 + co['w1'][2] * t1 + co['w2'][2] * t2

    # split big tiles into chunks <= CMAX, keeping a tile id for host merge
    chunks = []           # (tile_id, ty, tx, fidx)
    for tid, (ty, tx, fidx) in enumerate(tiles):
        n = len(fidx)
        nch = -(-n // CMAX)
        per = -(-n // nch)
        for j in range(0, n, per):
            chunks.append((tid, ty, tx, fidx[j:j + per]))

    # sort chunks by count desc, deal round-robin; slot pad = octet max
    chunks.sort(key=lambda ch: -len(ch[3]))
    nchunk = len(chunks)
    S = -(-nchunk // NCORES)
    cpads0 = []
    for s in range(S):
        grp = chunks[8 * s: 8 * s + 8]
        cmax = max(len(ch[3]) for ch in grp)
        cpads0.append(max(CGRAN, -(-cmax // CGRAN) * CGRAN))
    # reorder slots so PSUM containers are contiguous slot ranges
    order = _slot_order(cpads0)
    cpads = [cpads0[o] for o in order]
    groups = _pack_groups(cpads)
    gsum = [sum(cpads[s] for s in g) for g in groups]
    goff4 = np.concatenate([[0], np.cumsum([4 * t for t in gsum])]).astype(int)
    TOTC = int(goff4[-1])
    gsplit = max(1, len(groups) // 4)
    t0w = int(goff4[gsplit])
    # per-slot (group, bank-offset-base, within-group offset)
    slot_place = {}
    for gi, gr in enumerate(groups):
        off = 0
        for s in gr:
            slot_place[s] = (gi, int(goff4[gi]), off, gsum[gi])
            off += cpads[s]

    pp = np.arange(128)
    pxl = (pp % TW) - (TW / 2 - 0.5)          # -7.5 .. 7.5
    pyl = (pp // TW) - (TH / 2 - 0.5)         # -3.5 .. 3.5

    # stationary matmul operand [9,128]: rows [px*3, py*3, 1*3]
    pixlhs = np.stack([pxl, pxl, pxl, pyl, pyl, pyl,
                       np.ones(128), np.ones(128), np.ones(128)]).astype(bf16)
    assert np.all(pixlhs[0].astype(np.float64) == pxl)
    assert np.all(pixlhs[3].astype(np.float64) == pyl)

    c_lo, r_lo = grid['c_lo'], grid['r_lo']
    banks = ['w0', 'w1', 'w2', 'nd']

    in_maps = []
    rows_of = np.zeros((NCORES, S, 128), dtype=np.int32)
    cols_of = np.zeros((NCORES, S, 128), dtype=np.int32)
    real_of = np.zeros((NCORES, S, 128), dtype=bool)
    tile_of = np.full((NCORES, S), -1, dtype=np.int32)
    faces_of = [[None] * S for _ in range(NCORES)]
    for k in range(NCORES):
        coefs = np.zeros((9, TOTC), dtype=bf16)
        for s in range(S):
            c = cpads[s]
            gi, gbase, goff, gtot = slot_place[s]
            ci = 8 * order[s] + k
            if ci < nchunk:
                tid, ty, tx, fidx = chunks[ci]
                n = len(fidx)
                sx = c_lo + tx * TW + TW / 2.0
                sy = r_lo + ty * TH + TH / 2.0
                gx = c_lo + tx * TW + (pp % TW)
                gy = r_lo + ty * TH + (pp // TW)
                real = (gx <= IMAGE_SIZE - 1) & (gy <= IMAGE_SIZE - 1)
                rows_of[k, s] = np.minimum(gy, IMAGE_SIZE - 1)
                cols_of[k, s] = np.minimum(gx, IMAGE_SIZE - 1)
                real_of[k, s] = real
                tile_of[k, s] = tid
                faces_of[k][s] = fidx
            else:
                n = 0
            # bank-major coefficient columns [w0 c | w1 c | w2 c | nd c]
            for g in range(4):
                if g < 3:
                    a, b, cc = (v.copy() for v in co[banks[g]])
                    a = a * BIG; b = b * BIG; cc = cc * BIG
                else:
                    a, b, cc = (v.copy() for v in co['nd'])
                if n:
                    av = a[fidx]; bv = b[fidx]
                    cv = cc[fidx] + av * sx + bv * sy
                    av = np.concatenate([av, np.zeros(c - n)])
                    bv = np.concatenate([bv, np.zeros(c - n)])
                    cv = np.concatenate([cv, np.full(c - n, BAD if g == 3 else 0.0)])
                else:
                    av = np.zeros(c); bv = np.zeros(c)
                    cv = np.full(c, BAD if g == 3 else 0.0)
                lo = gbase + g * gtot + goff
                coefs[:, lo: lo + c] = _bank_rows9(av, bv, cv)
        im = dict(pixlhs=pixlhs, coefs0=coefs[:, :t0w],
                  coefs1=coefs[:, t0w:] if t0w < TOTC else np.zeros((9, 4), dtype=bf16))
        if im["coefs1"].shape[1] < 4:
            pad = np.zeros((9, 4), dtype=bf16)
            pad[:, :im["coefs1"].shape[1]] = im["coefs1"]
            im["coefs1"] = pad
        in_maps.append(im)

    scatter = dict(rows_of=rows_of, cols_of=cols_of, real_of=real_of,
                   tile_of=tile_of, faces_of=faces_of, colABC=colABC,
                   S=S, nchunk=nchunk)
    return cpads, in_maps, scatter


def assemble(results, scatter):
    out = np.zeros((1, 3, IMAGE_SIZE, IMAGE_SIZE), dtype=np.float32)
    S = scatter['S']
    tile_of = scatter['tile_of']
    colABC = scatter['colABC']
    bestcmx = np.full((IMAGE_SIZE, IMAGE_SIZE), -np.inf, dtype=np.float32)
    for k in range(NCORES):
        cmx = results[k]['cmxout']                        # [128, S]
        idx = results[k]['idxout']                        # [128, S]
        for s in range(S):
            if tile_of[k, s] < 0:
                continue
            fidx = scatter['faces_of'][k][s]
            n = len(fidx)
            valid = (cmx[:, s] > THRESH) & scatter['real_of'][k, s]
            if not np.any(valid):
                continue
            wi = np.minimum(idx[valid, s].astype(np.int64), n - 1)
            fglob = fidx[wi]
            rr = scatter['rows_of'][k, s][valid]
            cc = scatter['cols_of'][k, s][valid]
            cm = cmx[valid, s]
            upd = cm > bestcmx[rr, cc]
            if not np.any(upd):
                continue
            rr = rr[upd]; cc = cc[upd]
            bestcmx[rr, cc] = cm[upd]
            A = colABC[fglob[upd]]                        # [m, 9]
            px = cc + 0.5
            py = rr + 0.5
            for ch in range(3):
                out[0, ch, rr, cc] = (A[:, 3 * ch] * px + A[:, 3 * ch + 1] * py
                                      + A[:, 3 * ch + 2]).astype(np.float32)
    return out


def kernel(**inputs):
    from concourse.bass_utils import run_bass_kernel_spmd

    prep = prepare(**inputs)
    if prep is None:
        return np.zeros((1, 3, IMAGE_SIZE, IMAGE_SIZE), dtype=np.float32)
    cpads, in_maps, scatter = prep
    nc = _get_program(cpads)
    res = run_bass_kernel_spmd(nc, in_maps, core_ids=list(range(NCORES)))
    return assemble(res.results, scatter)


if __name__ == "__main__":
    pass
